# revision 1
# baseline (speedup 1.0000x reference)
"""Trainium2 Bass kernel for nn_ExtractRelevantPatchesLayer.

Per-image: 64x64 avg-pool on a [1024,1024] heatmap -> top-32 of the 256
pooled values -> gather the corresponding 64x64x3 image patches.

Sharding: batch dim (16) data-parallel across 8 NeuronCores, 2 images per
core, no cross-core communication.

Bit-exactness strategy (pooled values have 1-ULP gaps, so patch selection
must reproduce jax's f32 summation order exactly):
  - jax mean = sequential f32 sum over the 64 contiguous columns, then
    sequential f32 sum over the 64 rows (verified bitwise vs jax CPU+TRN).
  - DVE tensor_reduce is sequential over the free axis (HW-verified).
  - PE transpose (identity matmul) moves f32 bitwise (HW-verified), letting
    a second DVE reduce do the row sums sequentially.
  - top-32 via 4 rounds of max8/max_index/match_replace reproduces
    jax.lax.top_k ordering including duplicate handling (HW-verified).
Only the selected patches are read from HBM (dma_gather of 192-float rows),
so HBM traffic/core is ~8 MiB heatmap + 3 MiB gather + 3 MiB store.
"""
import os
import sys

for p in ("/opt/trn_rl_repo", "/root/.axon_site/_ro/trn_rl_repo"):
    if os.path.isdir(p) and p not in sys.path:
        sys.path.append(p)

import numpy as np

import concourse.bacc as bacc
import concourse.bass_isa as bass_isa
import concourse.mybir as mybir
import concourse.tile as tile
from concourse.tile_rust import add_dep_helper as _add_dep
from concourse.bass_utils import run_bass_kernel_spmd

F32 = mybir.dt.float32
I32 = mybir.dt.int32
I16 = mybir.dt.int16
U32 = mybir.dt.uint32

B_LOCAL = 2          # batches per core
N_CORES = 8
P = 64               # patch size
K = 32               # patches kept per batch
GRID = 16            # 16x16 candidate patches
NEG_FILL = -1.0e30

_CACHE: dict = {}


def _build_module():
    nc = bacc.Bacc("TRN2", target_bir_lowering=False, debug=False)

    # Local shard tensors (per core): 2 batches.
    hm_d = nc.dram_tensor("hm", [B_LOCAL * 1024, 1024], F32, kind="ExternalInput")
    img_d = nc.dram_tensor("img", [B_LOCAL * 16384, 192], F32, kind="ExternalInput")
    sel_d = nc.dram_tensor("sel", [B_LOCAL * K, P * P * 3], F32, kind="ExternalOutput")

    # Inline constants.
    ident_d = nc.inline_tensor(np.eye(128, dtype=np.float32), name="ident")
    ones_d = nc.inline_tensor(np.ones((128, 128), np.float32), name="ones")
    pp, ss = np.meshgrid(np.arange(128), np.arange(128), indexing="ij")
    # iotaA[p, s] = (p%16)*16 + (s%4)*256   (s = j*4 + rr_hi)
    iota_d = nc.inline_tensor(
        ((pp % 16) * 16 + (ss % 4) * 256).astype(np.float32), name="iotaA")
    # per-partition candidate index n(q, m) = 32*(q//16) + 16*m + q%16
    q = np.arange(128)
    n_qm = (32 * (q[:, None] // 16) + 16 * np.arange(2)[None, :]
            + (q[:, None] % 16))                        # [128, 2]
    # ltmask_m[q, f] = 1.0 if f < n(q, m)  (stable-rank tie term)
    f = np.arange(256)
    lt_np = (f[None, None, :] < n_qm[:, :, None]).astype(np.float32)  # [128,2,256]
    lt0_d = nc.inline_tensor(lt_np[:, 0, :].copy(), name="lt0")
    lt1_d = nc.inline_tensor(lt_np[:, 1, :].copy(), name="lt1")
    # rbase[q, m] = image row-block base of patch n(q, m)
    rbase_np = (n_qm + 1008 * (n_qm >> 4)).astype(np.float32)         # [128, 2]
    rbase_d = nc.inline_tensor(rbase_np, name="rbase")
    # jconst[p, j] = j  for slot-match
    jconst_d = nc.inline_tensor(
        np.tile(np.arange(32, dtype=np.float32), (128, 1)), name="jconst")

    with tile.TileContext(nc) as tc:
        with tc.tile_pool(name="consts", bufs=1) as cpool, \
             tc.tile_pool(name="heat", bufs=6) as hpool, \
             tc.tile_pool(name="work", bufs=1) as wpool, \
             tc.tile_pool(name="gath", bufs=1) as gpool, \
             tc.tile_pool(name="dr", bufs=1, space="DRAM") as dpool, \
             tc.tile_pool(name="ps", bufs=2, space="PSUM") as ppool:

            ident = cpool.tile([128, 128], F32, tag="ident", name="ident")
            nc.scalar.dma_start(ident[:], ident_d[:])
            ones = cpool.tile([128, 128], F32, tag="ones", name="ones")
            nc.scalar.dma_start(ones[:], ones_d[:])
            iota = cpool.tile([128, 128], F32, tag="iota", name="iota")
            nc.scalar.dma_start(iota[:], iota_d[:])
            lts = cpool.tile([128, 512], F32, tag="lts", name="lts")
            nc.scalar.dma_start(lts[:, 0:256], lt0_d[:])
            nc.scalar.dma_start(lts[:, 256:512], lt1_d[:])
            rbase = cpool.tile([128, 2], F32, tag="rbase", name="rbase")
            nc.scalar.dma_start(rbase[:], rbase_d[:])
            jconst = cpool.tile([128, 32], F32, tag="jconst", name="jconst")
            nc.scalar.dma_start(jconst[:], jconst_d[:])

            # Column partials: P_all[b][p, t*16+gw] = seq-sum over the 64
            # cols of group gw, row (t*128+p) of batch b.
            p_all = [wpool.tile([128, 128], F32, tag=f"pall{b}", name=f"pall{b}") for b in range(B_LOCAL)]

            def load_and_reduce(t):
                ht = hpool.tile([128, 1024], F32, tag="heat", name="heat")
                nc.sync.dma_start(ht[:], hm_d[t * 128:(t + 1) * 128, :])
                red = nc.vector.tensor_reduce(
                    out=p_all[t // 8][:, (t % 8) * 16:((t % 8) + 1) * 16],
                    in_=ht[:].rearrange("p (g c) -> p g c", c=64),
                    axis=mybir.AxisListType.X,
                    op=mybir.AluOpType.add,
                )
                return red

            def sums_to_vrep(b):
                # Row sums: transpose partials so each partition holds one
                # (t, gw) column of 128 row-partials, then reduce per 64.
                pt = ppool.tile([128, 128], F32, tag="pt", name="pt")
                nc.tensor.transpose(pt[:], p_all[b][:], ident[:])
                # Padded to 32 free elems so the second PE transpose is legal.
                sums = wpool.tile([128, 32], F32, tag=f"sums{b}", name=f"sums{b}")
                nc.gpsimd.memset(sums[:], 0.0)
                nc.vector.tensor_reduce(
                    out=sums[:, 0:2],
                    in_=pt[:].rearrange("q (m r) -> q m r", r=64),
                    axis=mybir.AxisListType.X,
                    op=mybir.AluOpType.add,
                )
                # vrep[p, n] = pooled sum of patch n (= 32t+16m+g), on every
                # partition: transpose -> rows m, per-m partition_broadcast of
                # the contiguous (t, g) row, then DVE strided interleave.
                # No DMA anywhere in this chain.
                pt2 = ppool.tile([32, 128], F32, tag="pt2", name="pt2")
                nc.tensor.transpose(pt2[:], sums[:], ident[:])
                s2 = wpool.tile([32, 128], F32, tag=f"s2{b}", name=f"s2{b}")
                nc.vector.tensor_copy(s2[:], pt2[:])
                # partition_broadcast sources must start at partition 0, so
                # stream_shuffle row m=1 up to partition 0 of a second tile.
                s2s = wpool.tile([32, 128], F32, tag=f"s2s{b}", name=f"s2s{b}")
                nc.vector.stream_shuffle(
                    s2s[0:32, :], s2[0:32, :], mask=[1] + list(range(1, 32)))
                vrep = wpool.tile([128, 256], F32, tag=f"vrep{b}", name=f"vrep{b}")
                vrep_v = vrep[:].rearrange("p (t m g) -> p t m g", t=8, m=2, g=16)
                for m, src in ((0, s2), (1, s2s)):
                    half = wpool.tile(
                        [128, 128], F32, tag=f"half{b}{m}", name=f"half{b}{m}")
                    nc.gpsimd.partition_broadcast(
                        half[:], src[0:1, :], channels=128)
                    nc.gpsimd.tensor_copy(
                        vrep_v[:, :, m],
                        half[:].rearrange("p (t g) -> p t g", g=16))
                return vrep, sums

            def batch_tail(b, vrep, sums):
                # Stable rank of each candidate (q, m) against all 256 pooled
                # sums: rank = #{v > v_n} + #{ties at lower n}. Matches
                # jax.lax.top_k ordering exactly (all-integer f32 math).
                rk = wpool.tile([128, 2], F32, tag=f"rk{b}", name=f"rk{b}")
                r2 = wpool.tile([128, 2], F32, tag=f"r2{b}", name=f"r2{b}")
                scratch = wpool.tile(
                    [128, 256], F32, tag=f"scr{b}", name=f"scr{b}")
                for m in range(2):
                    nc.vector.tensor_scalar(
                        scratch[:], vrep[:], sums[:, m:m + 1], 0.0,
                        op0=mybir.AluOpType.is_gt,
                        op1=mybir.AluOpType.add,
                        accum_out=rk[:, m:m + 1])
                    nc.vector.scalar_tensor_tensor(
                        out=scratch[:], in0=vrep[:], scalar=sums[:, m:m + 1],
                        in1=lts[:, m * 256:(m + 1) * 256],
                        op0=mybir.AluOpType.is_equal,
                        op1=mybir.AluOpType.mult,
                        accum_out=r2[:, m:m + 1])
                nc.vector.tensor_add(rk[:], rk[:], r2[:])

                # One-hot slot matrix scaled by rbase, then ones.T @ ZR
                # replicates the per-slot row-base across all partitions.
                zr = wpool.tile([128, 64], F32, tag=f"zr{b}", name=f"zr{b}")
                nc.vector.tensor_scalar(
                    zr[:, 0:32], jconst[:], rk[:, 0:1], None,
                    op0=mybir.AluOpType.is_equal)
                nc.vector.tensor_scalar(
                    zr[:, 32:64], jconst[:], rk[:, 1:2], None,
                    op0=mybir.AluOpType.is_equal)
                nc.vector.tensor_scalar(
                    zr[:, 0:32], zr[:, 0:32], rbase[:, 0:1], None,
                    op0=mybir.AluOpType.mult)
                nc.vector.scalar_tensor_tensor(
                    out=zr[:, 0:32], in0=zr[:, 32:64], scalar=rbase[:, 1:2],
                    in1=zr[:, 0:32],
                    op0=mybir.AluOpType.mult, op1=mybir.AluOpType.add)
                rbs = ppool.tile([128, 32], F32, tag="rbs", name="rbs")
                nc.tensor.matmul(
                    out=rbs[:], lhsT=ones[:], rhs=zr[:, 0:32],
                    start=True, stop=True)

                # k = rbs[slot] + rr_hi*256 + q*16, converted to int16.
                krows = wpool.tile(
                    [128, 128], F32, tag=f"krows{b}", name=f"krows{b}")
                kr_inst = nc.vector.tensor_add(
                    krows[:].rearrange("p (j h) -> p j h", h=4),
                    iota[:].rearrange("p (j h) -> p j h", h=4),
                    rbs[:].to_broadcast([128, 32, 4]))
                idx16 = wpool.tile([128, 128], I16, tag=f"k16{b}", name=f"k16{b}")
                nc.gpsimd.tensor_copy(idx16[:], krows[:])

                # Gather the 2048 patch rows (192 f32 each) of this batch.
                gath = gpool.tile([128, 16 * 192], F32, tag=f"g{b}", name=f"g{b}")
                nc.gpsimd.dma_gather(
                    out_ap=gath[:].rearrange("p (m c) -> p m c", c=192),
                    in_ap=img_d[b * 16384:(b + 1) * 16384, :],
                    idxs_ap=idx16[:],
                    num_idxs=2048,
                    num_idxs_reg=2048,
                    elem_size=192,
                    single_packet=False,
                )
                # Store: gathered row g=j*64+rr sits at [64*(j%2)+rr, j//2].
                sel_v = sel_d[:].rearrange(
                    "(bb jh jl) (r c) -> bb jl r jh c", bb=B_LOCAL, jh=16, jl=2, c=192)
                for jl in range(2):
                    nc.sync.dma_start(
                        sel_v[b, jl],
                        gath[jl * 64:(jl + 1) * 64, :].rearrange(
                            "p (m c) -> p m c", c=192),
                    )
                return kr_inst

            # Emission order = scheduler priority: batch 0's entire tail
            # outranks batch 1's loads/reduces, so b0's gather DMA is ready
            # the moment the heatmap stream drains.
            for t in range(8):
                load_and_reduce(t)
            kr0 = batch_tail(0, *sums_to_vrep(0))
            late_reds = []
            for t in range(8, 16):
                red = load_and_reduce(t)
                if t >= 12:
                    late_reds.append(red)
            # Keep DVE clear for batch 0's rank chain: the last four batch-1
            # reduces wait until b0's final DVE op so its gather can be
            # enqueued the moment the heatmap stream drains.
            for red in late_reds:
                _add_dep(red.ins, kr0.ins,
                         reason="pipeline: late b1 reduces yield to b0 rank chain")
            batch_tail(1, *sums_to_vrep(1))

    nc.compile()
    return nc


def _get_module():
    if "nc" not in _CACHE:
        _CACHE["nc"] = _build_module()
    return _CACHE["nc"]


LAST_RESULTS = None  # BassKernelResults of the most recent kernel() call


def kernel(heatmap, image):
    global LAST_RESULTS
    heatmap = np.ascontiguousarray(np.asarray(heatmap), dtype=np.float32)
    image = np.ascontiguousarray(np.asarray(image), dtype=np.float32)
    B = heatmap.shape[0]
    assert B == B_LOCAL * N_CORES

    nc = _get_module()
    in_maps = []
    for c in range(N_CORES):
        hm = heatmap[c * B_LOCAL:(c + 1) * B_LOCAL].reshape(B_LOCAL * 1024, 1024)
        im = image[c * B_LOCAL:(c + 1) * B_LOCAL].reshape(B_LOCAL * 16384, 192)
        in_maps.append({"hm": hm, "img": im})

    trace = os.environ.get("KERNEL_PROFILE", "") == "1"
    try:
        res = run_bass_kernel_spmd(
            nc, in_maps, core_ids=list(range(N_CORES)), trace=trace)
    except ModuleNotFoundError:
        # NTFF profiling hook unavailable in this environment
        res = run_bass_kernel_spmd(
            nc, in_maps, core_ids=list(range(N_CORES)), trace=False)
    LAST_RESULTS = res
    out = np.concatenate(
        [res.results[c]["sel"].reshape(B_LOCAL * K, P, P, 3) for c in range(N_CORES)],
        axis=0)
    return out



# revision 13
# speedup vs baseline: 1.1368x; 1.1368x over previous
"""Trainium2 Bass kernel for nn_ExtractRelevantPatchesLayer.

Per-image: 64x64 avg-pool on a [1024,1024] heatmap -> top-32 of the 256
pooled values -> gather the corresponding 64x64x3 image patches.

Sharding: batch dim (16) data-parallel across 8 NeuronCores, 2 images per
core, no cross-core communication.

Bit-exactness strategy (pooled values have 1-ULP gaps and exact ties, so
patch selection must reproduce jax's f32 summation order and top_k tie
order exactly):
  - jax mean = sequential f32 sum over the 64 contiguous columns, then
    sequential f32 sum over the 64 rows (verified bitwise vs jax CPU+TRN).
  - DVE tensor_reduce is sequential over the free axis (HW-verified).
  - PE transpose and ones@diag(v) matmuls move f32 bitwise: each output
    accumulates exactly one 1.0*v product plus zeros.
  - top-32 via stable rank: rank(n) = #{v_f > v_n} + #{f < n : v_f == v_n}
    reproduces jax.lax.top_k ordering including duplicate handling.

Layout: the host pre-arranges the image so every candidate patch is two
contiguous 24 KiB half-patch rows ([1024, 6144] per core-batch-pair), so
each gather is 32 large descriptors per half-patch set instead of 2048
row-sized ones.  All constants are generated on-chip (iota /
affine_select / memset); the DMA engines move only heatmap (8 MiB),
gathered patches (3 MiB) and stores (3 MiB) per core.

Schedule: the single per-core DMA pipe is the bottleneck (360 GB/s in the
cost model).  Heatmap tiles stream back-to-back; batch 0's rank chain
runs on Pool/Act (keeping DVE clear for batch 1's tile reduces), batch
1's latency-critical chain runs on DVE/PE right after its last reduce,
with the rank passes split across DVE (m=0) and Pool (m=1).  Gathers and
stores are split in half so the DMA tail interleaves
g0a,g0b,s0a,s0b,g1a,g1b,s1a,s1b with minimal idle.
"""
import os
import sys

for p in ("/opt/trn_rl_repo", "/root/.axon_site/_ro/trn_rl_repo"):
    if os.path.isdir(p) and p not in sys.path:
        sys.path.append(p)

import numpy as np

import concourse.bacc as bacc
import concourse.mybir as mybir
import concourse.tile as tile
from concourse.tile_rust import add_dep_helper as _add_dep
from concourse.bass_utils import run_bass_kernel_spmd

F32 = mybir.dt.float32
I32 = mybir.dt.int32
I16 = mybir.dt.int16

B_LOCAL = 2          # batches per core
N_CORES = 8
P = 64               # patch size
K = 32               # patches kept per batch

_CACHE: dict = {}

OP = mybir.AluOpType
AX = mybir.AxisListType


def _build_module():
    nc = bacc.Bacc("TRN2", target_bir_lowering=False, debug=False)

    # Local shard tensors (per core): 2 batches.
    hm_d = nc.dram_tensor("hm", [B_LOCAL * 1024, 1024], F32, kind="ExternalInput")
    # Patch-contiguous image: row 2*(256*b + n) + h = half h of patch n.
    img_d = nc.dram_tensor("img", [B_LOCAL * 512, 6144], F32, kind="ExternalInput")
    sel_d = nc.dram_tensor("sel", [B_LOCAL * K, P * P * 3], F32, kind="ExternalOutput")

    with tile.TileContext(nc) as tc:
        with tc.tile_pool(name="consts", bufs=1) as cpool, \
             tc.tile_pool(name="heat", bufs=6) as hpool, \
             tc.tile_pool(name="work", bufs=1) as wpool, \
             tc.tile_pool(name="gath", bufs=1) as gpool, \
             tc.tile_pool(name="ps", bufs=1, space="PSUM") as ppool:

            # ---------------- on-chip constants (no DMA) ----------------
            ones = cpool.tile([128, 128], F32, tag="ones", name="ones")
            nc.gpsimd.memset(ones[:], 1.0)
            # ident[p, f] = 1 iff f == p   (affine iota f - p == 0)
            ident = cpool.tile([128, 128], F32, tag="ident", name="ident")
            nc.gpsimd.affine_select(
                ident[:], ones[:], [[1, 128]], OP.is_equal, 0.0,
                base=0, channel_multiplier=-1)
            # jconst[p, j] = j
            ji = cpool.tile([128, 32], I32, tag="ji", name="ji")
            nc.gpsimd.iota(ji[:], [[1, 32]], base=0, channel_multiplier=0)
            jconst = cpool.tile([128, 32], F32, tag="jconst", name="jconst")
            nc.gpsimd.tensor_copy(jconst[:], ji[:])
            # n(q, m) = 32*(q//16) + 16*m + q%16 and the idx-slot owner
            # pscal[q, c] = 16c + q%16 need q%16, which no legal on-chip op
            # produces -- one 4 KiB inline-const DMA (~56 ns) covers both.
            q = np.arange(128)
            cst_np = np.zeros((128, 8), np.float32)
            cst_np[:, 0] = 2 * q - (q % 16)           # n(q, 0)
            cst_np[:, 1] = 2 * q - (q % 16) + 16      # n(q, 1)
            cst_np[:, 2] = q % 16                     # pscal[:, 0]
            cst_np[:, 3] = (q % 16) + 16              # pscal[:, 1]
            cst_d = nc.inline_tensor(cst_np, name="cst")
            cst = cpool.tile([128, 8], F32, tag="cst", name="cst")
            nc.scalar.dma_start(cst[:], cst_d[:])
            nqm = cst[:, 0:2]
            pscal = cst[:, 2:4]
            # msel[q, c, j] = 2 * (j == pscal[q, c])   (x2 folds the
            # half-patch row doubling of the gather index into the mask)
            msel = cpool.tile([128, 2, 32], F32, tag="msel", name="msel")
            for c in range(2):
                nc.gpsimd.tensor_scalar(
                    msel[:, c, :], jconst[:], pscal[:, c:c + 1], 2.0,
                    op0=OP.is_equal, op1=OP.mult)
            # nrow[p, m*128 + q] = n(q, m) on every partition, via
            # ones^T @ (ident * nqm[:, m]) -- exact (one 1*v + zeros).
            znr = wpool.tile([128, 256], F32, tag="znr", name="znr")
            for m in range(2):
                nc.vector.tensor_scalar(
                    znr[:, m * 128:(m + 1) * 128], ident[:],
                    nqm[:, m:m + 1], None, op0=OP.mult)
            nrow = ppool.tile([128, 256], F32, tag="vrep", name="nrow")
            nc.tensor.matmul(out=nrow[:], lhsT=ones[:], rhs=znr[:],
                             start=True, stop=True)
            # lts[q, m*256 + c] = 1 iff n(col c) < n(q, m)  (tie-break mask)
            lts = cpool.tile([128, 512], F32, tag="lts", name="lts")
            for m in range(2):
                nc.vector.tensor_scalar(
                    lts[:, m * 256:(m + 1) * 256], nrow[:],
                    nqm[:, m:m + 1], None, op0=OP.is_lt)

            # ---------------- heatmap streaming ----------------
            p_all = [wpool.tile([128, 128], F32, tag=f"pall{b}",
                                name=f"pall{b}") for b in range(B_LOCAL)]

            def load_reduce(t, cols=None):
                # tile t covers hm rows [128t, 128t+128); cols=(lo, hi)
                # loads/reduces only that column range (64-col groups).
                ht = hpool.tile([128, 1024], F32, tag="heat", name="heat")
                lo, hi = cols if cols else (0, 1024)
                nc.sync.dma_start(ht[:, lo:hi], hm_d[t * 128:(t + 1) * 128, lo:hi])
                return nc.vector.tensor_reduce(
                    out=p_all[t // 8][:, (t % 8) * 16 + lo // 64:
                                      (t % 8) * 16 + hi // 64],
                    in_=ht[:, lo:hi].rearrange("p (g c) -> p g c", c=64),
                    axis=AX.X,
                    op=OP.add,
                )

            def transpose_b(b):
                pt = ppool.tile([128, 128], F32, tag="pt", name=f"pt{b}")
                nc.tensor.transpose(pt[:], p_all[b][:], ident[:])
                return pt

            def sums_vrep(b, pt, ZE):
                """Row sums (DVE, sequential = bitwise) then broadcast to all
                partitions via ones^T @ diag-scatter (exact).  ZE: engine for
                the diag scatter (Pool for b0 keeps DVE clear)."""
                sums = wpool.tile([128, 2], F32, tag=f"sums{b}", name=f"sums{b}")
                red2 = nc.vector.tensor_reduce(
                    out=sums[:],
                    in_=pt[:].rearrange("q (m r) -> q m r", r=64),
                    axis=AX.X, op=OP.add)
                z = wpool.tile([128, 256], F32, tag=f"z{b}", name=f"z{b}")
                zi = [ZE.tensor_scalar(
                          z[:, m * 128:(m + 1) * 128], ident[:],
                          sums[:, m:m + 1], None, op0=OP.mult)
                      for m in range(2)]
                vrep = ppool.tile([128, 256], F32, tag="vrep", name=f"vrep{b}")
                for m in range(2):
                    nc.tensor.matmul(out=vrep[:, m * 128:(m + 1) * 128],
                                     lhsT=ones[:], rhs=z[:, m * 128:(m + 1) * 128],
                                     start=True, stop=True)
                return sums, vrep, red2, zi

            def rank_ops(E, V, sums, rk, r2, scr, m):
                i1 = E.tensor_scalar(
                    scr[:], V, sums[:, m:m + 1], 0.0,
                    op0=OP.is_gt, op1=OP.add,
                    accum_out=rk[:, m:m + 1])
                i2 = E.scalar_tensor_tensor(
                    out=scr[:], in0=V, scalar=sums[:, m:m + 1],
                    in1=lts[:, m * 256:(m + 1) * 256],
                    op0=OP.is_equal, op1=OP.mult,
                    accum_out=r2[:, m:m + 1])
                return i1, i2

            def tail(b, E, sums, vrep, vrep_pool):
                """rank -> slot scatter -> gather indices -> gathers.
                E = nc.vector (b1): everything on DVE reading PSUM directly
                (it is free once its reduces end).  E = nc.gpsimd (b0): Pool
                does the plain compare/mask work on the Act-copied SBUF vrep
                (m=1) and all small ops; DVE only gets the ops Pool cannot
                run (accumulates / free-axis reduces), pinned into heat
                stream gaps by the caller."""
                rk = wpool.tile([128, 2], F32, tag=f"rk{b}", name=f"rk{b}")
                scr = wpool.tile([128, 256], F32, tag=f"scr{b}", name=f"scr{b}")
                dve = {}
                if E is nc.vector:
                    r2 = wpool.tile([128, 2], F32, tag=f"r2{b}", name=f"r2{b}")
                    for m in range(2):
                        rank_ops(nc.vector, vrep[:], sums, rk, r2, scr, m)
                    rkt = wpool.tile([128, 2], F32, tag=f"rkt{b}",
                                     name=f"rkt{b}")
                    nc.vector.tensor_add(rkt[:], rk[:], r2[:])
                else:
                    # m=0: the two DVE accumulate passes (illegal on Pool).
                    r2 = wpool.tile([128, 2], F32, tag=f"r2{b}", name=f"r2{b}")
                    dve["rank0"] = rank_ops(nc.vector, vrep[:], sums, rk, r2,
                                            scr, 0)
                    # m=1: plain Pool compare/mask/add, then one DVE reduce.
                    sA = wpool.tile([128, 256], F32, tag=f"sA{b}", name=f"sA{b}")
                    sB = wpool.tile([128, 256], F32, tag=f"sB{b}", name=f"sB{b}")
                    nc.gpsimd.tensor_scalar(
                        sA[:], vrep_pool, sums[:, 1:2], None, op0=OP.is_gt)
                    nc.gpsimd.tensor_scalar(
                        sB[:], vrep_pool, sums[:, 1:2], None, op0=OP.is_equal)
                    nc.gpsimd.tensor_mul(sB[:], sB[:], lts[:, 256:512])
                    nc.gpsimd.tensor_add(sA[:], sA[:], sB[:])
                    rkt = wpool.tile([128, 2], F32, tag=f"rkt{b}",
                                     name=f"rkt{b}")
                    dve["rkred1"] = nc.vector.tensor_reduce(
                        out=rkt[:, 1:2],
                        in_=sA[:].rearrange("p (x c) -> p x c", x=1),
                        axis=AX.X, op=OP.add)
                    nc.gpsimd.tensor_add(rkt[:, 0:1], rk[:, 0:1], r2[:, 0:1])
                # zr[q, j] = sum_m (rkt[q,m] == j) * n(q,m); the two ones^T
                # matmuls accumulate the m-halves in PSUM (exact: one 1*v
                # product per column plus zeros).
                zr = wpool.tile([128, 64], F32, tag=f"zr{b}", name=f"zr{b}")
                for m in range(2):
                    E.tensor_scalar(
                        zr[:, m * 32:(m + 1) * 32], jconst[:],
                        rkt[:, m:m + 1], nqm[:, m:m + 1],
                        op0=OP.is_equal, op1=OP.mult)
                rbs = ppool.tile([128, 32], F32, tag="rbs", name=f"rbs{b}")
                nc.tensor.matmul(out=rbs[:], lhsT=ones[:], rhs=zr[:, 0:32],
                                 start=True, stop=False)
                nc.tensor.matmul(out=rbs[:], lhsT=ones[:], rhs=zr[:, 32:64],
                                 start=False, stop=True)
                # nsel[q, c] = 2 * n_of_rank(16c + q%16)
                nsel = wpool.tile([128, 2], F32, tag=f"nsel{b}", name=f"nsel{b}")
                if E is nc.vector:
                    scr32 = wpool.tile([128, 32], F32, tag=f"s32{b}",
                                       name=f"s32{b}")
                    for c in range(2):
                        nc.vector.scalar_tensor_tensor(
                            out=scr32[:], in0=rbs[:], scalar=1.0,
                            in1=msel[:, c, :],
                            op0=OP.mult, op1=OP.mult,
                            accum_out=nsel[:, c:c + 1])
                else:
                    rbss = wpool.tile([128, 32], F32, tag=f"rbss{b}",
                                      name=f"rbss{b}")
                    nc.scalar.copy(rbss[:], rbs[:])
                    scr32 = wpool.tile([128, 2, 32], F32, tag=f"s32{b}",
                                       name=f"s32{b}")
                    for c in range(2):
                        nc.gpsimd.tensor_mul(
                            scr32[:, c, :], rbss[:], msel[:, c, :])
                    dve["nselred"] = nc.vector.tensor_reduce(
                        out=nsel[:], in_=scr32[:],
                        axis=AX.X, op=OP.add)
                # idxab[:, 0:2] = 2n (upper half rows), [:, 2:4] = 2n+1;
                # each idx op is emitted right before the gather it feeds so
                # the first descriptor gen starts as early as possible.
                idxab = wpool.tile([128, 4], I16, tag=f"idx{b}", name=f"idx{b}")
                gath = gpool.tile([128, 2, 6144], F32, tag=f"g{b}", name=f"g{b}")
                nc.gpsimd.tensor_copy(idxab[:, 0:2], nsel[:])
                for h in range(2):
                    nc.gpsimd.dma_gather(
                        out_ap=gath[:, h:h + 1, :],
                        in_ap=img_d[b * 512:(b + 1) * 512, :],
                        idxs_ap=idxab[:, 2 * h:2 * h + 2],
                        num_idxs=K,
                        num_idxs_reg=K,
                        elem_size=6144,
                        single_packet=False,
                    )
                    if h == 0:
                        nc.gpsimd.tensor_scalar(
                            idxab[:, 2:4], nsel[:], 1.0, None, op0=OP.add)
                return gath, dve

            def store(b, gath):
                sel_v = sel_d[:].rearrange(
                    "(bb j) (h c) -> bb j h c", bb=B_LOCAL, h=2)
                for h in range(2):
                    nc.sync.dma_start(sel_v[b, :, h], gath[0:32, h, :])

            # ---- emission order = per-engine SEQ order + priority ----
            for t in range(7):
                load_reduce(t)
            load_reduce(7, cols=(0, 512))
            load_reduce(7, cols=(512, 1024))
            pt0 = transpose_b(0)
            sums0, vrep0, red2_0, z0 = sums_vrep(0, pt0, nc.gpsimd)
            vs0 = wpool.tile([128, 256], F32, tag="vs0", name="vs0")
            nc.scalar.copy(vs0[:], vrep0[:])
            g0, dve0 = tail(0, nc.gpsimd, sums0, vrep0, vrep_pool=vs0[:])
            r8 = load_reduce(8)
            r9 = load_reduce(9)
            r10 = load_reduce(10)
            r11 = load_reduce(11)
            r12 = load_reduce(12)
            r13 = load_reduce(13)
            load_reduce(14)
            load_reduce(15, cols=(0, 512))
            load_reduce(15, cols=(512, 1024))
            pt1 = transpose_b(1)
            sums1, vrep1, _, _ = sums_vrep(1, pt1, nc.vector)
            g1, _ = tail(1, nc.vector, sums1, vrep1, vrep_pool=None)
            store(0, g0)
            store(1, g1)
            # Pin b0's few DVE ops into heat-stream data-wait gaps so b1's
            # tile reduces are never pushed past their DMA arrival times.
            _add_dep(r8.ins, red2_0.ins,
                     reason="pipeline: b0 row-sums before b1 reduces")
            _add_dep(dve0["rank0"][0].ins, r9.ins,
                     reason="pipeline: b0 rank-m0 after t9 reduce")
            _add_dep(r10.ins, dve0["rank0"][1].ins,
                     reason="pipeline: t10 reduce after b0 rank-m0")
            _add_dep(dve0["rkred1"].ins, r10.ins,
                     reason="pipeline: b0 rank-m1 reduce after t10")
            _add_dep(r11.ins, dve0["rkred1"].ins,
                     reason="pipeline: t11 reduce after b0 rank-m1 reduce")
            _add_dep(dve0["nselred"].ins, r12.ins,
                     reason="pipeline: b0 nsel reduce after t12")
            _add_dep(r13.ins, dve0["nselred"].ins,
                     reason="pipeline: t13 reduce after b0 nsel reduce")

    nc.compile()
    return nc


def _get_module():
    if "nc" not in _CACHE:
        _CACHE["nc"] = _build_module()
    return _CACHE["nc"]


LAST_RESULTS = None  # BassKernelResults of the most recent kernel() call


def kernel(heatmap, image):
    global LAST_RESULTS
    heatmap = np.ascontiguousarray(np.asarray(heatmap), dtype=np.float32)
    image = np.ascontiguousarray(np.asarray(image), dtype=np.float32)
    B = heatmap.shape[0]
    assert B == B_LOCAL * N_CORES

    nc = _get_module()
    in_maps = []
    for c in range(N_CORES):
        hm = heatmap[c * B_LOCAL:(c + 1) * B_LOCAL].reshape(B_LOCAL * 1024, 1024)
        # patch-contiguous halves: row 2*(256b + gh*16 + gw) + r//32
        im = (image[c * B_LOCAL:(c + 1) * B_LOCAL]
              .reshape(B_LOCAL, 16, 64, 16, 64 * 3)
              .transpose(0, 1, 3, 2, 4)
              .reshape(B_LOCAL * 512, 6144))
        in_maps.append({"hm": hm, "img": np.ascontiguousarray(im)})

    trace = os.environ.get("KERNEL_PROFILE", "") == "1"
    try:
        res = run_bass_kernel_spmd(
            nc, in_maps, core_ids=list(range(N_CORES)), trace=trace)
    except ModuleNotFoundError:
        # NTFF profiling hook unavailable in this environment
        res = run_bass_kernel_spmd(
            nc, in_maps, core_ids=list(range(N_CORES)), trace=False)
    LAST_RESULTS = res
    out = np.concatenate(
        [res.results[c]["sel"].reshape(B_LOCAL * K, P, P, 3)
         for c in range(N_CORES)],
        axis=0)
    return out
